# revision 1
# baseline (speedup 1.0000x reference)
"""Causal self-attention (B=2, S=2048, D=1024, H=16) on 8 TRN2 NeuronCores.

Sharding: sequence-parallel. Core c handles batch c//4 and query rows
[512*(c%4), 512*(c%4)+512). Each core computes the full K/V projection for
its batch (redundant 4x, but avoids collectives entirely), attention for
its query block against all 2048 keys (causal handled by an additive mask),
and the output projection for its rows. Outputs are disjoint -> host concat.

All matmuls run in float32r (TF32-like, full PE rate at N>=256).

Dataflow per core:
  phase 0: Q^T = 0.125*(Wq^T xq + bq) -> SBUF resident    [8][128,512]
  phase 1: stream x^T by token chunk; K^T -> DRAM [8][128,2048],
           V (+ones col) -> DRAM chunk-major [16][128, 16*65]
  phase 2: per head: S^T = K^T.T Q^T (per 128-kv chunk), +mask (DVE),
           exp (ACT, scale=0.125) -> P^T f32r; O^T[65,512] += [V|1].T P^T
           (row 64 = softmax denominators); per-head normalize via
           reciprocal + K=1 ones broadcast matmul
  phase 3: out = y @ Wp + bp  (per 128-token chunk), DMA out
"""

import os
import numpy as np

import concourse.bass as bass
import concourse.mybir as mybir
import concourse.tile as tile
from concourse import bacc
from concourse.bass_utils import run_bass_kernel_spmd

F32 = mybir.dt.float32
F32R = mybir.dt.float32r
AF = mybir.ActivationFunctionType
ALU = mybir.AluOpType

B, S, D, H, HD = 2, 2048, 1024, 16, 64
QL = 512          # query rows per core
NKC = D // 128    # 8 model-dim chunks
NHP = H // 2      # 8 head pairs
KVC = S // 128    # 16 kv chunks
NEG = -1.0e9
SCALE = 1.0 / np.sqrt(HD)

_CACHED = {}


def build_nc():
    nc = bacc.Bacc("TRN2", target_bir_lowering=False, debug=False)

    # ---- I/O ----
    xt = nc.dram_tensor("xt", [D, S], F32R, kind="ExternalInput").ap()
    xqt = nc.dram_tensor("xqt", [D, QL], F32R, kind="ExternalInput").ap()
    wa = nc.dram_tensor("wa", [D, 3 * D], F32R, kind="ExternalInput").ap()
    ba = nc.dram_tensor("ba", [3 * D], F32, kind="ExternalInput").ap()
    wp = nc.dram_tensor("wp", [D, D], F32R, kind="ExternalInput").ap()
    bp = nc.dram_tensor("bp", [1, D], F32R, kind="ExternalInput").ap()
    bav = nc.dram_tensor("bav", [1, D], F32R, kind="ExternalInput").ap()
    maskt = nc.dram_tensor("maskt", [S, QL], F32, kind="ExternalInput").ap()
    sel = nc.dram_tensor("sel", [2, 128], F32R, kind="ExternalInput").ap()
    ones1 = nc.dram_tensor("ones1", [1, 128], F32R, kind="ExternalInput").ap()
    out = nc.dram_tensor("out", [QL, D], F32, kind="ExternalOutput").ap()

    with tile.TileContext(nc) as tc:
        _body(nc, tc, xt, xqt, wa, ba, wp, bp, bav, maskt, sel, ones1, out)
    nc.compile()
    return nc


def _body(nc, tc, xt, xqt, wa, ba, wp, bp, bav, maskt, sel, ones1, out):
    # ---------- persistent pools ----------
    with (
        tc.tile_pool(name="const", bufs=1) as const_p,
        tc.tile_pool(name="qt", bufs=1) as qt_p,
        tc.tile_pool(name="psum", bufs=5, space="PSUM") as psum,
        tc.tile_pool(name="dram", bufs=1, space="DRAM") as dram,
    ):
        # ---------- constants ----------
        ones_s = const_p.tile([1, 128], F32R)
        nc.sync.dma_start(ones_s[:], ones1[:])
        bp_s = const_p.tile([1, D], F32R)
        nc.sync.dma_start(bp_s[:], bp[:])
        bav_s = const_p.tile([1, D], F32R)
        nc.sync.dma_start(bav_s[:], bav[:])
        baq = [const_p.tile([128, 1], F32, tag=f"baq{i}", name=f"baq{i}")
               for i in range(NHP)]
        bak = [const_p.tile([128, 1], F32, tag=f"bak{i}", name=f"bak{i}")
               for i in range(NHP)]
        for hp in range(NHP):
            nc.sync.dma_start(
                baq[hp][:], ba[hp * 128:(hp + 1) * 128].rearrange("(p o) -> p o", o=1)
            )
            nc.sync.dma_start(
                bak[hp][:],
                ba[D + hp * 128:D + (hp + 1) * 128].rearrange("(p o) -> p o", o=1),
            )
        # broadcast v-bias and proj-bias to all 128 partitions via K=1 matmul
        bav_bc = const_p.tile([128, D], F32)
        bp_bc = const_p.tile([128, D], F32)
        for n2 in range(2):
            t = psum.tile([128, 512], F32, tag="ps", name=f"bc{n2}")
            nc.tensor.matmul(
                t[:], ones_s[:], bav_s[:, n2 * 512:(n2 + 1) * 512],
                start=True, stop=True,
            )
            nc.vector.tensor_copy(bav_bc[:, n2 * 512:(n2 + 1) * 512], t[:])
            t2 = psum.tile([128, 512], F32, tag="ps", name=f"bc2{n2}")
            nc.tensor.matmul(
                t2[:], ones_s[:], bp_s[:, n2 * 512:(n2 + 1) * 512],
                start=True, stop=True,
            )
            nc.vector.tensor_copy(bp_bc[:, n2 * 512:(n2 + 1) * 512], t2[:])

        qt_s = [qt_p.tile([128, QL], F32R, tag=f"qt{hp}", name=f"qt{hp}")
                for hp in range(NHP)]
        kt_d = dram.tile([NHP, 128, S], F32R)
        v_d = dram.tile([KVC, 128, H * 65], F32R)

        # ---------- phases 0+1: QKV projection ----------
        TCH = 256  # token chunk width for streaming x^T
        NTC = S // TCH
        with (
            tc.tile_pool(name="waw", bufs=1) as wa_p,
            tc.tile_pool(name="xq", bufs=1) as xq_p,
            tc.tile_pool(name="xtc", bufs=2) as xtc_p,
            tc.tile_pool(name="vsb", bufs=2) as vsb_p,
            tc.tile_pool(name="ktsb", bufs=3) as ktsb_p,
        ):
            wa_s = [wa_p.tile([128, 3 * D], F32R, tag=f"wa{kc}", name=f"wa{kc}")
                    for kc in range(NKC)]
            for kc in range(NKC):
                nc.sync.dma_start(wa_s[kc][:], wa[kc * 128:(kc + 1) * 128, :])
            xq_s = [xq_p.tile([128, QL], F32R, tag=f"xq{kc}", name=f"xq{kc}")
                    for kc in range(NKC)]
            for kc in range(NKC):
                nc.sync.dma_start(xq_s[kc][:], xqt[kc * 128:(kc + 1) * 128, :])

            # Q^T
            for hp in range(NHP):
                ps = psum.tile([128, QL], F32, tag="ps", name=f"q{hp}")
                for kc in range(NKC):
                    nc.tensor.matmul(
                        ps[:],
                        wa_s[kc][:, hp * 128:(hp + 1) * 128],
                        xq_s[kc][:],
                        start=(kc == 0),
                        stop=(kc == NKC - 1),
                    )
                # q = 0.125*psum + (0.125*bq)  (host prescaled bias)
                nc.vector.tensor_scalar(
                    qt_s[hp][:], ps[:], SCALE, baq[hp][:], ALU.mult, ALU.add
                )

            # K^T and V per token chunk
            for tcn in range(NTC):
                xt_c = [
                    xtc_p.tile([128, TCH], F32R, tag=f"xt{kc}", name=f"xtc{kc}_{tcn}")
                    for kc in range(NKC)
                ]
                for kc in range(NKC):
                    nc.sync.dma_start(
                        xt_c[kc][:],
                        xt[kc * 128:(kc + 1) * 128, tcn * TCH:(tcn + 1) * TCH],
                    )
                for hp in range(NHP):
                    ps = psum.tile([128, TCH], F32, tag="ps", name=f"k{hp}_{tcn}")
                    for kc in range(NKC):
                        nc.tensor.matmul(
                            ps[:],
                            wa_s[kc][:, D + hp * 128:D + (hp + 1) * 128],
                            xt_c[kc][:],
                            start=(kc == 0),
                            stop=(kc == NKC - 1),
                        )
                    kt_sb = ktsb_p.tile(
                        [128, TCH], F32R, tag="ktsb", name=f"ktsb{hp}_{tcn}"
                    )
                    nc.vector.tensor_scalar(
                        kt_sb[:], ps[:], bak[hp][:], None, ALU.add
                    )
                    nc.sync.dma_start(
                        kt_d[hp, :, tcn * TCH:(tcn + 1) * TCH], kt_sb[:]
                    )
                # V natural, with ones column at stride-65 slot 64
                for vt in range(TCH // 128):
                    vtg = tcn * (TCH // 128) + vt
                    v_sb = vsb_p.tile([128, H * 65], F32R, tag="vsb", name=f"vsb{vtg}")
                    v3 = v_sb[:].rearrange("p (h e) -> p h e", e=65)
                    for nh in range(2):
                        ps = psum.tile([128, 512], F32, tag="ps", name=f"v{vtg}_{nh}")
                        for kc in range(NKC):
                            nc.tensor.matmul(
                                ps[:],
                                xt_c[kc][:, vt * 128:(vt + 1) * 128],
                                wa_s[kc][:, 2 * D + nh * 512:2 * D + (nh + 1) * 512],
                                start=(kc == 0),
                                stop=(kc == NKC - 1),
                            )
                        nc.vector.tensor_tensor(
                            v3[:, nh * 8:(nh + 1) * 8, 0:64],
                            ps[:].rearrange("p (h e) -> p h e", e=64),
                            bav_bc[:, nh * 512:(nh + 1) * 512].rearrange(
                                "p (h e) -> p h e", e=64
                            ),
                            ALU.add,
                        )
                    nc.vector.memset(v3[:, :, 64:65].bitcast(F32), 1.0)
                    nc.sync.dma_start(v_d[vtg], v_sb[:])

        # ---------- phase 2: attention ----------
        with tc.tile_pool(name="yt", bufs=1) as yt_p:
            with (
                tc.tile_pool(name="mask", bufs=1) as mask_p,
                tc.tile_pool(name="kth", bufs=2) as kth_p,
                tc.tile_pool(name="vh", bufs=3) as vh_p,
                tc.tile_pool(name="pt", bufs=4) as p_p,
                tc.tile_pool(name="den", bufs=1) as den_p,
                tc.tile_pool(name="opsum", bufs=2, space="PSUM") as opsum,
            ):
                mask_s = mask_p.tile([128, KVC * QL], F32)
                for kvc in range(KVC):
                    nc.sync.dma_start(
                        mask_s[:, kvc * QL:(kvc + 1) * QL],
                        maskt[kvc * 128:(kvc + 1) * 128, :],
                    )
                yt_s = [yt_p.tile([128, QL], F32, tag=f"yt{hp}", name=f"yt{hp}")
                        for hp in range(NHP)]
                ytr_s = [yt_p.tile([128, QL], F32R, tag=f"ytr{hp}", name=f"ytr{hp}")
                         for hp in range(NHP)]

                for h in range(H):
                    hp, hh = h // 2, h % 2
                    if hh == 0:
                        kt_h = kth_p.tile(
                            [128, S], F32R, tag="kth", name=f"kth{hp}"
                        )
                        nc.sync.dma_start(kt_h[:], kt_d[hp])
                        kt_cur = kt_h
                    v_h = vh_p.tile([128, KVC * 65], F32R, tag="vh", name=f"vh{h}")
                    nc.sync.dma_start(
                        v_h[:].rearrange("p (t e) -> p t e", e=65),
                        v_d[:, :, h * 65:(h + 1) * 65].rearrange("t p e -> p t e"),
                    )
                    op = opsum.tile([65, QL], F32, tag="o", name=f"op{h}")
                    for kvc in range(KVC):
                        sp = psum.tile([128, QL], F32, tag="ps", name=f"s{h}_{kvc}")
                        nc.tensor.matmul(
                            sp[:],
                            kt_cur[hh * 64:(hh + 1) * 64, kvc * 128:(kvc + 1) * 128],
                            qt_s[hp][hh * 64:(hh + 1) * 64, :],
                            start=True,
                            stop=True,
                        )
                        nc.vector.tensor_tensor(
                            sp[:], sp[:], mask_s[:, kvc * QL:(kvc + 1) * QL], ALU.add
                        )
                        pt = p_p.tile([128, QL], F32R, tag="p", name=f"pt{h}_{kvc}")
                        nc.scalar.activation(pt[:], sp[:], AF.Exp)
                        nc.tensor.matmul(
                            op[:],
                            v_h[:, kvc * 65:(kvc + 1) * 65],
                            pt[:],
                            start=(kvc == 0),
                            stop=(kvc == KVC - 1),
                        )
                    den_h = den_p.tile([1, QL], F32, tag="den", name=f"den{h}", bufs=3)
                    nc.vector.tensor_copy(den_h[:], op[64:65, :])
                    nc.vector.tensor_copy(
                        yt_s[hp][hh * 64:(hh + 1) * 64, :], op[0:64, :]
                    )
                    rec_h = den_p.tile([1, QL], F32R, tag="rec", name=f"rec{h}", bufs=3)
                    with nc.allow_low_precision(reason="f32r denominators"):
                        nc.vector.reciprocal(rec_h[:], den_h[:])
                    rp = psum.tile([128, QL], F32, tag="ps", name=f"rp{h}")
                    nc.tensor.matmul(
                        rp[0:64, :], ones_s[:, 0:64], rec_h[:], start=True, stop=True
                    )
                    nc.vector.tensor_tensor(
                        ytr_s[hp][hh * 64:(hh + 1) * 64, :],
                        yt_s[hp][hh * 64:(hh + 1) * 64, :],
                        rp[0:64, :],
                        ALU.mult,
                    )

            # ---------- phase 3: out projection ----------
            with (
                tc.tile_pool(name="wpp", bufs=1) as wp_p,
                tc.tile_pool(name="outp", bufs=3) as out_p,
            ):
                wp_s = [wp_p.tile([128, D], F32R, tag=f"wp{kc}", name=f"wps{kc}")
                        for kc in range(NKC)]
                for kc in range(NKC):
                    nc.sync.dma_start(wp_s[kc][:], wp[kc * 128:(kc + 1) * 128, :])
                for tc4 in range(4):
                    for n2 in range(2):
                        ps = psum.tile([128, 512], F32, tag="ps", name=f"o{tc4}_{n2}")
                        for hp in range(NHP):
                            nc.tensor.matmul(
                                ps[:],
                                ytr_s[hp][:, tc4 * 128:(tc4 + 1) * 128],
                                wp_s[hp][:, n2 * 512:(n2 + 1) * 512],
                                start=(hp == 0),
                                stop=(hp == NHP - 1),
                            )
                        ot = out_p.tile([128, 512], F32, tag="ot", name=f"ot{tc4}_{n2}")
                        nc.vector.tensor_tensor(
                            ot[:], ps[:], bp_bc[:, n2 * 512:(n2 + 1) * 512], ALU.add
                        )
                        nc.sync.dma_start(
                            out[tc4 * 128:(tc4 + 1) * 128, n2 * 512:(n2 + 1) * 512],
                            ot[:],
                        )


def _host_inputs(x, w_attn, b_attn, w_proj, b_proj):
    """Build the 8 per-core input maps."""
    ba = b_attn.astype(np.float32).copy()
    ba[:D] *= SCALE
    sel = np.zeros((2, 128), np.float32)
    sel[0, 0:64] = 1.0
    sel[1, 64:128] = 1.0
    ones1 = np.ones((1, 128), np.float32)
    bp1 = b_proj.astype(np.float32).reshape(1, D)
    bav1 = b_attn[2 * D:3 * D].astype(np.float32).reshape(1, D)
    wa = np.ascontiguousarray(w_attn.astype(np.float32))
    wpp = np.ascontiguousarray(w_proj.astype(np.float32))

    in_maps = []
    for c in range(8):
        b, cq = c // 4, c % 4
        xb = x[b]  # [S, D]
        xt = np.ascontiguousarray(xb.T)  # [D, S]
        xqt = np.ascontiguousarray(xb[cq * QL:(cq + 1) * QL].T)  # [D, QL]
        # maskt[j, i] = 0 if j <= cq*QL + i else NEG
        j = np.arange(S)[:, None]
        i = np.arange(QL)[None, :]
        maskt = np.where(j <= cq * QL + i, 0.0, NEG).astype(np.float32)
        in_maps.append(
            {
                "xt": xt,
                "xqt": xqt,
                "wa": wa,
                "ba": ba,
                "wp": wpp,
                "bp": bp1,
                "bav": bav1,
                "maskt": maskt,
                "sel": sel,
                "ones1": ones1,
            }
        )
    return in_maps


def kernel(x, w_attn, b_attn, w_proj, b_proj):
    x = np.asarray(x, np.float32)
    w_attn = np.asarray(w_attn, np.float32)
    b_attn = np.asarray(b_attn, np.float32)
    w_proj = np.asarray(w_proj, np.float32)
    b_proj = np.asarray(b_proj, np.float32)

    if "nc" not in _CACHED:
        _CACHED["nc"] = build_nc()
    nc = _CACHED["nc"]
    in_maps = _host_inputs(x, w_attn, b_attn, w_proj, b_proj)
    res = run_bass_kernel_spmd(nc, in_maps, core_ids=list(range(8)))
    outs = [res.results[c]["out"] for c in range(8)]
    full = np.empty((B, S, D), np.float32)
    for c in range(8):
        b, cq = c // 4, c % 4
        full[b, cq * QL:(cq + 1) * QL] = outs[c]
    return full



# revision 4
# speedup vs baseline: 30065.0777x; 30065.0777x over previous
"""Causal self-attention (B=2, S=2048, D=1024, H=16) on 8 TRN2 NeuronCores.

Sharding: tensor-parallel over heads x data-parallel over batch.
Core c handles batch c//4 and head group c%4 (4 heads = 2 head-pairs),
computing Q/K/V projections for its 768 qkv columns over ALL 2048 tokens,
causal attention for its 4 heads over all queries, and a PARTIAL output
projection (its 256 rows of w_proj). The host sums the 4 partials per
batch and adds b_proj (the all-reduce of the TP decomposition, done on
host since outputs are disjoint in no other way).

All big matmuls run in bf16 (full PE rate + fast weight load); PSUM
accumulation is f32. Causality is exploited at 128-kv-chunk granularity:
fully-masked chunks are skipped, the 4 diagonal chunks per 512-query tile
are computed on a shrinking query range (N = 512-128j) with only a
[128,128] triangle mask each.

Per-core dataflow:
  phase A: Q^T,K^T (per head-pair, [128,2048] bf16 SBUF-resident),
           V (+ones col) chunk-major [128, 16*130] per head-pair
  phase B: per (head-pair, 512-query tile): scores S^T = K^T.T Q^T per
           128-kv chunk (two heads on disjoint PE row halves), exp on ACT
           (batched pairs of chunks), O^T[65,512] += [V|1].T P^T
           (row 64 = softmax denominator); normalize via reciprocal +
           K=1 ones-broadcast matmul
  phase C: partial out = y @ wp_rows (bf16), f32 out, DMA
"""

import numpy as np
import ml_dtypes

import concourse.bass as bass
import concourse.mybir as mybir
import concourse.tile as tile
from concourse import bacc
from concourse.bass_utils import run_bass_kernel_spmd

F32 = mybir.dt.float32
F32R = mybir.dt.float32r
BF16 = mybir.dt.bfloat16
AF = mybir.ActivationFunctionType
ALU = mybir.AluOpType

B, S, D, H, HD = 2, 2048, 1024, 16, 64
HPC = 2            # head-pairs per core (4 heads)
GC = 256           # qkv columns per core per projection
NKC = D // 128     # 8 model-dim chunks
KVC = S // 128     # 16 kv chunks
NQT = S // 512     # 4 query tiles
NEG = -1.0e9
SCALE = 1.0 / np.sqrt(HD)

_CACHED = {}


def build_nc():
    nc = bacc.Bacc("TRN2", target_bir_lowering=False, debug=False)

    xt = nc.dram_tensor("xt", [D, S], BF16, kind="ExternalInput").ap()
    wg = nc.dram_tensor("wg", [D, 3 * GC], BF16, kind="ExternalInput").ap()
    wpg = nc.dram_tensor("wpg", [GC, D], BF16, kind="ExternalInput").ap()
    baq = nc.dram_tensor("baq", [GC], F32, kind="ExternalInput").ap()
    bak = nc.dram_tensor("bak", [GC], F32, kind="ExternalInput").ap()
    bav = nc.dram_tensor("bav", [1, GC], F32R, kind="ExternalInput").ap()
    tri = nc.dram_tensor("tri", [128, 128], F32, kind="ExternalInput").ap()
    ones1 = nc.dram_tensor("ones1", [1, 128], F32R, kind="ExternalInput").ap()
    out = nc.dram_tensor("out", [S, D], F32, kind="ExternalOutput").ap()

    with tile.TileContext(nc) as tc:
        _body(nc, tc, xt, wg, wpg, baq, bak, bav, tri, ones1, out)
    nc.compile()
    return nc


def _body(nc, tc, xt, wg, wpg, baq, bak, bav, tri, ones1, out):
    with (
        tc.tile_pool(name="const", bufs=1) as const_p,
        tc.tile_pool(name="w", bufs=1) as w_p,
        tc.tile_pool(name="qkv", bufs=1) as qkv_p,
    ):
        # ---------- constants ----------
        ones_s = const_p.tile([1, 128], F32R)
        nc.sync.dma_start(ones_s[:], ones1[:])
        bav_s = const_p.tile([1, GC], F32R)
        nc.sync.dma_start(bav_s[:], bav[:])
        tri_s = const_p.tile([128, 128], F32)
        nc.sync.dma_start(tri_s[:], tri[:])
        baq_t = [const_p.tile([128, 1], F32, tag=f"baq{i}", name=f"baq{i}")
                 for i in range(HPC)]
        bak_t = [const_p.tile([128, 1], F32, tag=f"bak{i}", name=f"bak{i}")
                 for i in range(HPC)]
        for hp in range(HPC):
            nc.sync.dma_start(
                baq_t[hp][:],
                baq[hp * 128:(hp + 1) * 128].rearrange("(p o) -> p o", o=1),
            )
            nc.sync.dma_start(
                bak_t[hp][:],
                bak[hp * 128:(hp + 1) * 128].rearrange("(p o) -> p o", o=1),
            )
        bav_bc = const_p.tile([128, GC], F32)

        # ---------- weights + x^T ----------
        wg_s = [w_p.tile([128, 3 * GC], BF16, tag=f"wg{kc}", name=f"wg{kc}")
                for kc in range(NKC)]
        xt_s = [w_p.tile([128, S], BF16, tag=f"xt{kc}", name=f"xts{kc}")
                for kc in range(NKC)]
        for kc in range(NKC):
            nc.sync.dma_start(wg_s[kc][:], wg[kc * 128:(kc + 1) * 128, :])
            nc.sync.dma_start(xt_s[kc][:], xt[kc * 128:(kc + 1) * 128, :])

        # ---------- persistent qkv / y ----------
        qt_s = [qkv_p.tile([128, S], BF16, tag=f"qt{hp}", name=f"qt{hp}")
                for hp in range(HPC)]
        kt_s = [qkv_p.tile([128, S], BF16, tag=f"kt{hp}", name=f"kt{hp}")
                for hp in range(HPC)]
        v_s = [qkv_p.tile([128, KVC * 130], BF16, tag=f"v{hp}", name=f"v{hp}")
               for hp in range(HPC)]
        ytr_s = [qkv_p.tile([128, S], BF16, tag=f"ytr{hp}", name=f"ytr{hp}")
                 for hp in range(HPC)]

        # ---------- phase A: projections ----------
        with tc.tile_pool(name="psA", bufs=4, space="PSUM") as psA:
            # broadcast v-bias to all 128 partitions via K=1 matmul
            bps = psA.tile([128, GC], F32, tag="pa", name="bavbc")
            nc.tensor.matmul(bps[:], ones_s[:], bav_s[:], start=True, stop=True)
            nc.vector.tensor_copy(bav_bc[:], bps[:])

            for hp in range(HPC):
                for t4 in range(NQT):
                    ps = psA.tile([128, 512], F32, tag="pa", name=f"q{hp}_{t4}")
                    for kc in range(NKC):
                        nc.tensor.matmul(
                            ps[:],
                            wg_s[kc][:, hp * 128:(hp + 1) * 128],
                            xt_s[kc][:, t4 * 512:(t4 + 1) * 512],
                            start=(kc == 0),
                            stop=(kc == NKC - 1),
                        )
                    nc.vector.tensor_scalar(
                        qt_s[hp][:, t4 * 512:(t4 + 1) * 512], ps[:],
                        SCALE, baq_t[hp][:], ALU.mult, ALU.add,
                    )
                for t4 in range(NQT):
                    ps = psA.tile([128, 512], F32, tag="pa", name=f"k{hp}_{t4}")
                    for kc in range(NKC):
                        nc.tensor.matmul(
                            ps[:],
                            wg_s[kc][:, GC + hp * 128:GC + (hp + 1) * 128],
                            xt_s[kc][:, t4 * 512:(t4 + 1) * 512],
                            start=(kc == 0),
                            stop=(kc == NKC - 1),
                        )
                    nc.vector.tensor_scalar(
                        kt_s[hp][:, t4 * 512:(t4 + 1) * 512], ps[:],
                        bak_t[hp][:], None, ALU.add,
                    )
            for c in range(KVC):
                ps = psA.tile([128, GC], F32, tag="pa", name=f"v{c}")
                for kc in range(NKC):
                    nc.tensor.matmul(
                        ps[:],
                        xt_s[kc][:, c * 128:(c + 1) * 128],
                        wg_s[kc][:, 2 * GC:3 * GC],
                        start=(kc == 0),
                        stop=(kc == NKC - 1),
                    )
                for hp in range(HPC):
                    v3 = v_s[hp][:, c * 130:(c + 1) * 130].rearrange(
                        "p (h e) -> p h e", e=65
                    )
                    nc.vector.tensor_tensor(
                        v3[:, :, 0:64],
                        ps[:, hp * 128:(hp + 1) * 128].rearrange(
                            "p (h e) -> p h e", e=64
                        ),
                        bav_bc[:, hp * 128:(hp + 1) * 128].rearrange(
                            "p (h e) -> p h e", e=64
                        ),
                        ALU.add,
                    )
                    nc.vector.memset(v3[:, :, 64:65], 1.0)

        # ---------- phase B: attention ----------
        with (
            tc.tile_pool(name="scp", bufs=3, space="PSUM") as sc_p,
            tc.tile_pool(name="opp", bufs=2, space="PSUM") as op_p,
            tc.tile_pool(name="ptp", bufs=6) as pt_p,
            tc.tile_pool(name="den", bufs=3) as den_p,
        ):
            for hp in range(HPC):
                for t in range(NQT):
                    ops = [
                        op_p.tile([65, 512], F32, tag="op", name=f"op{hp}_{t}_{hh}")
                        for hh in range(2)
                    ]
                    nfull = 4 * t
                    qsl = qt_s[hp]

                    def s_mm(sc, dst_off, n, hh, c, q_off):
                        nc.tensor.matmul(
                            sc[:, dst_off:dst_off + n],
                            kt_s[hp][hh * 64:(hh + 1) * 64,
                                     c * 128:(c + 1) * 128],
                            qsl[hh * 64:(hh + 1) * 64,
                                t * 512 + q_off:t * 512 + q_off + n],
                            start=True,
                            stop=True,
                        )

                    def pv_mm(sc_pt, src_off, n, hh, c, q_off):
                        nc.tensor.matmul(
                            ops[hh][:, q_off:512],
                            v_s[hp][:, c * 130 + hh * 65:c * 130 + hh * 65 + 65],
                            sc_pt[:, src_off:src_off + n],
                            start=(c == 0),
                            stop=(c == 4 * t + 3),
                        )

                    # full chunk pairs
                    for g0 in range(0, nfull, 2):
                        for hh in range(2):
                            sc = sc_p.tile(
                                [128, 1024], F32, tag="sc",
                                name=f"sc{hp}_{t}_{g0}_{hh}",
                            )
                            for dc in range(2):
                                s_mm(sc, dc * 512, 512, hh, g0 + dc, 0)
                            pt = pt_p.tile(
                                [128, 1024], BF16, tag="pt",
                                name=f"pt{hp}_{t}_{g0}_{hh}",
                            )
                            nc.scalar.activation(pt[:], sc[:], AF.Exp)
                            for dc in range(2):
                                pv_mm(pt, dc * 512, 512, hh, g0 + dc, 0)

                    # diagonal chunks, shrinking query range + triangle mask
                    for grp in range(2):
                        js = (0, 1) if grp == 0 else (2, 3)
                        for hh in range(2):
                            sc = sc_p.tile(
                                [128, 1024], F32, tag="sc",
                                name=f"scd{hp}_{t}_{grp}_{hh}",
                            )
                            off = 0
                            offs = []
                            for j in js:
                                n = 512 - 128 * j
                                s_mm(sc, off, n, hh, 4 * t + j, 128 * j)
                                offs.append((j, off, n))
                                off += n
                            for j, o, n in offs:
                                nc.vector.tensor_tensor(
                                    sc[:, o:o + 128], sc[:, o:o + 128],
                                    tri_s[:], ALU.add,
                                )
                            pt = pt_p.tile(
                                [128, 1024], BF16, tag="pt",
                                name=f"ptd{hp}_{t}_{grp}_{hh}",
                            )
                            nc.scalar.activation(pt[:, 0:off], sc[:, 0:off], AF.Exp)
                            for j, o, n in offs:
                                pv_mm(pt, o, n, hh, 4 * t + j, 128 * j)

                    # normalize: y^T = O^T[0:64] * (1/den) broadcast
                    for hh in range(2):
                        rec = den_p.tile(
                            [1, 512], F32R, tag="rec", name=f"rec{hp}_{t}_{hh}"
                        )
                        with nc.allow_low_precision(reason="f32r denominators"):
                            nc.vector.reciprocal(rec[:], ops[hh][64:65, :])
                        yt = den_p.tile(
                            [64, 512], F32, tag="yt", name=f"yt{hp}_{t}_{hh}"
                        )
                        nc.any.tensor_copy(yt[:], ops[hh][0:64, :])
                        rp = sc_p.tile(
                            [64, 512], F32, tag="sc", name=f"rp{hp}_{t}_{hh}"
                        )
                        nc.tensor.matmul(
                            rp[:], ones_s[:, 0:64], rec[:], start=True, stop=True
                        )
                        nc.vector.tensor_tensor(
                            ytr_s[hp][hh * 64:(hh + 1) * 64,
                                      t * 512:(t + 1) * 512],
                            yt[:], rp[:], ALU.mult,
                        )

        # ---------- phase C: partial out-projection ----------
        with (
            tc.tile_pool(name="wpp", bufs=1) as wp_p,
            tc.tile_pool(name="pcp", bufs=3, space="PSUM") as psC,
            tc.tile_pool(name="outp", bufs=3) as out_p,
        ):
            wp_s = [wp_p.tile([128, D], BF16, tag=f"wp{hp}", name=f"wps{hp}")
                    for hp in range(HPC)]
            for hp in range(HPC):
                nc.sync.dma_start(wp_s[hp][:], wpg[hp * 128:(hp + 1) * 128, :])
            for ts in range(S // 128):
                for nh in range(2):
                    ps = psC.tile([128, 512], F32, tag="pc", name=f"o{ts}_{nh}")
                    for hp in range(HPC):
                        nc.tensor.matmul(
                            ps[:],
                            ytr_s[hp][:, ts * 128:(ts + 1) * 128],
                            wp_s[hp][:, nh * 512:(nh + 1) * 512],
                            start=(hp == 0),
                            stop=(hp == HPC - 1),
                        )
                    ot = out_p.tile([128, 512], F32, tag="ot", name=f"ot{ts}_{nh}")
                    nc.vector.tensor_copy(ot[:], ps[:])
                    nc.sync.dma_start(
                        out[ts * 128:(ts + 1) * 128, nh * 512:(nh + 1) * 512],
                        ot[:],
                    )


def _host_inputs(x, w_attn, b_attn, w_proj):
    bf16 = ml_dtypes.bfloat16
    j = np.arange(128)[:, None]
    i = np.arange(128)[None, :]
    tri = np.where(i >= j, 0.0, NEG).astype(np.float32)
    ones1 = np.ones((1, 128), np.float32)

    in_maps = []
    for c in range(8):
        b, g = c // 4, c % 4
        xt = np.ascontiguousarray(x[b].T).astype(bf16)
        cols = slice(g * GC, (g + 1) * GC)
        wg = np.concatenate(
            [w_attn[:, cols], w_attn[:, D:][:, cols], w_attn[:, 2 * D:][:, cols]],
            axis=1,
        ).astype(bf16)
        wpg = np.ascontiguousarray(w_proj[g * GC:(g + 1) * GC, :]).astype(bf16)
        baq = (b_attn[cols] * SCALE).astype(np.float32)
        bak = b_attn[D:][cols].astype(np.float32).copy()
        bav = b_attn[2 * D:][cols].astype(np.float32).reshape(1, GC)
        in_maps.append(
            {
                "xt": xt,
                "wg": wg,
                "wpg": wpg,
                "baq": baq,
                "bak": bak,
                "bav": bav,
                "tri": tri,
                "ones1": ones1,
            }
        )
    return in_maps


def kernel(x, w_attn, b_attn, w_proj, b_proj):
    x = np.asarray(x, np.float32)
    w_attn = np.asarray(w_attn, np.float32)
    b_attn = np.asarray(b_attn, np.float32)
    w_proj = np.asarray(w_proj, np.float32)
    b_proj = np.asarray(b_proj, np.float32)

    if "nc" not in _CACHED:
        _CACHED["nc"] = build_nc()
    nc = _CACHED["nc"]
    in_maps = _host_inputs(x, w_attn, b_attn, w_proj)
    res = run_bass_kernel_spmd(nc, in_maps, core_ids=list(range(8)))
    globals()["_LAST_RES"] = res
    full = np.empty((B, S, D), np.float32)
    for b in range(B):
        acc = res.results[4 * b]["out"].astype(np.float32)
        for g in range(1, 4):
            acc = acc + res.results[4 * b + g]["out"]
        full[b] = acc + b_proj[None, :]
    return full
